# revision 16
# baseline (speedup 1.0000x reference)
"""Trainium2 Bass kernel for CodeArt self-attention (B=4, S=2048, D=768, H=12).

Sharding: 8 cores = (batch b, head-group hg of 6 heads).  Each core computes
attention for its batch over its 6 heads and writes [S, 384] of the output.

Math (per core, per head h):
  qT/kT = (Wh x^T) in [64, S] layout (bf16), v in [S, 64] (f32) + ones column.
  S_T[k,q] = k^T q  (PSUM, f32)  += 8 * pwl_bias_h[bucket_T]  (identity matmul)
  E = exp(0.125*S_T + (a_h + mask[k]))          (ACT, per-partition bias)
  ctx_T_aug[65, q] = sum_k v_aug[k,65]^T E[k,q] (f32r matmul; row 64 = denom)
  out[q, 64] = transpose(ctx_T)/denom

The 8-entry rel_bias lookup t_h[bucket] (bucket = umin(uint32(rel), 7)) is
exact via a piecewise-linear basis {1, x, |x-1|, ..., |x-6|}; the 8x8 solve
for coefficients happens on host (data-dependent values flow in as tensors).
"""

import sys

sys.path.insert(0, "/opt/trn_rl_repo")

import numpy as np
import ml_dtypes

B, S, D, H = 4, 2048, 768, 12
HPC = 6  # heads per core
DH = 64
DAUG = 776  # 768 + 1 ones row + 7 pad rows (zeros)
NKT = 7  # contraction tiles over DAUG: 6*128 + 8
NKB = 16  # k blocks of 128
NQT = 4  # q tiles of 512
QT = 512
NJ = 6  # abs-basis terms |x-1|..|x-6|

_cache = {}


def _build():
    import concourse.bass as bass
    import concourse.bacc as bacc
    import concourse.tile as tile
    from concourse import mybir

    dt = mybir.dt
    f32, bf16, i32, u32 = dt.float32, dt.bfloat16, dt.int32, dt.uint32
    f32r = dt.float32r
    Alu = mybir.AluOpType
    Act = mybir.ActivationFunctionType

    nc = bacc.Bacc("TRN2", target_bir_lowering=False, debug=False)

    xT = nc.dram_tensor("xT", [DAUG, S], bf16, kind="ExternalInput")
    wqT = nc.dram_tensor("wqT", [DAUG, HPC * DH], bf16, kind="ExternalInput")
    wkT = nc.dram_tensor("wkT", [DAUG, HPC * DH], bf16, kind="ExternalInput")
    wvT = nc.dram_tensor("wvT", [DAUG, HPC * DH], bf16, kind="ExternalInput")
    rel = nc.dram_tensor("rel", [S, S], i32, kind="ExternalInput")
    coefs = nc.dram_tensor("coefs", [128, HPC * 8], f32, kind="ExternalInput")
    eb = nc.dram_tensor("eb", [128, NKB * HPC], f32, kind="ExternalInput")
    idb = nc.dram_tensor("idb", [128, 128], bf16, kind="ExternalInput")
    idf = nc.dram_tensor("idf", [128, 128], f32, kind="ExternalInput")
    y = nc.dram_tensor("y", [S, HPC * DH], f32, kind="ExternalOutput")

    with tile.TileContext(nc) as tc:
        with (
            tc.tile_pool(name="dram", bufs=1, space="DRAM") as dram,
            tc.tile_pool(name="persist", bufs=1) as persist,
        ):
            bounce = dram.tile([S, S], bf16)

            # ---- constants ----
            coefs_t = persist.tile([128, HPC * 8], f32, tag="coefs")
            nc.sync.dma_start(out=coefs_t, in_=coefs[:, :])
            eb_t = persist.tile([128, NKB * HPC], f32, tag="eb")
            nc.sync.dma_start(out=eb_t, in_=eb[:, :])
            idb_t = persist.tile([128, 128], bf16, tag="idb")
            nc.sync.dma_start(out=idb_t, in_=idb[:, :])
            idf_t = persist.tile([128, 128], f32, tag="idf")
            nc.sync.dma_start(out=idf_t, in_=idf[:, :])


            # ---- phase A: projections ----
            with (
                tc.tile_pool(name="phA", bufs=1) as phA,
                tc.tile_pool(name="work", bufs=2) as work,
                tc.tile_pool(name="psum_proj", bufs=1, space="PSUM") as psum_proj,
            ):
                xT_t = []
                for kt in range(NKT):
                    t = phA.tile([128, S], bf16, tag=f"xT{kt}")
                    rows = 128 if kt < 6 else DAUG - 6 * 128
                    nc.sync.dma_start(
                        out=t[:rows, :], in_=xT[kt * 128 : kt * 128 + rows, :]
                    )
                    xT_t.append(t)
                w_t = {}
                for name, wt in (("q", wqT), ("k", wkT), ("v", wvT)):
                    tiles = []
                    for kt in range(NKT):
                        t = phA.tile([128, HPC * DH], bf16, tag=f"w{name}{kt}")
                        rows = 128 if kt < 6 else DAUG - 6 * 128
                        nc.sync.dma_start(
                            out=t[:rows, :], in_=wt[kt * 128 : kt * 128 + rows, :]
                        )
                        tiles.append(t)
                    w_t[name] = tiles

                # qT, kT: [384, S] bf16 as 3 tiles of [128, S]
                qkT = {"q": [], "k": []}
                for name in ("q", "k"):
                    for dto in range(3):
                        ps = psum_proj.tile([128, S], mybir.dt.float32, tag="pproj")
                        for nt in range(NQT):
                            for kt in range(NKT):
                                rows = 128 if kt < 6 else DAUG - 6 * 128
                                nc.tensor.matmul(
                                    ps[:, nt * QT : (nt + 1) * QT],
                                    lhsT=w_t[name][kt][
                                        :rows, dto * 128 : (dto + 1) * 128
                                    ],
                                    rhs=xT_t[kt][:rows, nt * QT : (nt + 1) * QT],
                                    start=(kt == 0),
                                    stop=(kt == NKT - 1),
                                )
                        sb = persist.tile([128, S], bf16, tag=f"{name}T{dto}")
                        nc.scalar.copy(out=sb, in_=ps)
                        qkT[name].append(sb)

                # v: 16 tiles [128, 390] f32 — per head 65 cols (64 v + 1 ones)
                v_t = []
                for st in range(NKB):
                    ps = psum_proj.tile([128, HPC * DH], mybir.dt.float32, tag="pv")
                    for kt in range(NKT):
                        rows = 128 if kt < 6 else DAUG - 6 * 128
                        nc.tensor.matmul(
                            ps,
                            lhsT=xT_t[kt][:rows, st * 128 : (st + 1) * 128],
                            rhs=w_t["v"][kt][:rows, :],
                            start=(kt == 0),
                            stop=(kt == NKT - 1),
                        )
                    vt = persist.tile([128, HPC * 65], f32r, tag=f"v{st}")
                    v3 = vt.rearrange("p (h c) -> p h c", h=HPC)
                    nc.scalar.copy(
                        out=v3[:, :, 0:DH],
                        in_=ps.rearrange("p (h c) -> p h c", h=HPC),
                    )
                    nc.vector.memset(
                        v3[:, :, DH : DH + 1].bitcast(mybir.dt.float32), 1.0
                    )
                    v_t.append(vt)

                # bucket (natural) -> DRAM bounce, bf16
                for qb in range(NKB):
                    rt = work.tile([128, S], i32, tag="relt")
                    nc.sync.dma_start(out=rt, in_=rel[qb * 128 : (qb + 1) * 128, :])
                    bk = work.tile([128, S], bf16, tag="bkt")
                    nc.vector.tensor_scalar(
                        out=bk,
                        in0=rt.bitcast(u32),
                        scalar1=7,
                        scalar2=None,
                        op0=Alu.min,
                    )
                    nc.sync.dma_start(
                        out=bounce[qb * 128 : (qb + 1) * 128, :], in_=bk
                    )

            # ---- phase B ----
            with (
                tc.tile_pool(name="slabB", bufs=1) as slabB,
                tc.tile_pool(name="ework", bufs=3) as ework,
                tc.tile_pool(name="psum_s", bufs=3, space="PSUM") as psum_s,
                tc.tile_pool(name="psum_ctx", bufs=2, space="PSUM") as psum_ctx,
                tc.tile_pool(name="psum_tp", bufs=2, space="PSUM") as psum_tp,
            ):
                for qt in range(NQT):
                    # transposed bucket slab for this q-tile: [128k, QT] per kb
                    bTq = []
                    for kb in range(NKB):
                        t = slabB.tile([128, QT], bf16, tag=f"bTq{kb}")
                        nc.sync.dma_start_transpose(
                            t,
                            bounce[
                                qt * QT : (qt + 1) * QT, kb * 128 : (kb + 1) * 128
                            ],
                        )
                        bTq.append(t)
                    # abs basis slab
                    basis = []
                    for kb in range(NKB):
                        row = []
                        for j in range(1, NJ + 1):
                            t = slabB.tile([128, QT], bf16, tag=f"A{kb}_{j}")
                            nc.vector.tensor_scalar(
                                out=t,
                                in0=bTq[kb],
                                scalar1=float(j),
                                scalar2=0.0,
                                op0=Alu.subtract,
                                op1=Alu.max,
                            )
                            row.append(t)
                        basis.append(row)

                    for h in range(HPC):
                        hq = h * DH
                        kslice_t = qkT["k"][hq // 128]
                        qslice_t = qkT["q"][hq // 128]
                        po = hq % 128
                        ctx_ps = psum_ctx.tile([65, QT], mybir.dt.float32, tag="ctx")
                        for kb in range(NKB):
                            s_ps = psum_s.tile([128, QT], mybir.dt.float32, tag="s")
                            nc.tensor.matmul(
                                s_ps,
                                lhsT=kslice_t[po : po + DH, kb * 128 : (kb + 1) * 128],
                                rhs=qslice_t[po : po + DH, qt * QT : (qt + 1) * QT],
                                start=True,
                                stop=False,
                            )
                            # PWL bias combine (bf16): b = sum_j c_j |x-j| + cx*x
                            bt = ework.tile([128, QT], bf16, tag="bias")
                            nc.vector.tensor_scalar(
                                out=bt,
                                in0=basis[kb][0],
                                scalar1=coefs_t[:, h * 8 + 1 : h * 8 + 2],
                                scalar2=None,
                                op0=Alu.mult,
                            )
                            for j in range(2, NJ + 1):
                                nc.vector.scalar_tensor_tensor(
                                    out=bt,
                                    in0=basis[kb][j - 1],
                                    scalar=coefs_t[:, h * 8 + j : h * 8 + j + 1],
                                    in1=bt,
                                    op0=Alu.mult,
                                    op1=Alu.add,
                                )
                            nc.vector.scalar_tensor_tensor(
                                out=bt,
                                in0=bTq[kb],
                                scalar=coefs_t[:, h * 8 : h * 8 + 1],
                                in1=bt,
                                op0=Alu.mult,
                                op1=Alu.add,
                            )
                            nc.tensor.matmul(
                                s_ps,
                                lhsT=idb_t,
                                rhs=bt,
                                start=False,
                                stop=True,
                            )
                            et = ework.tile([128, QT], f32r, tag="e")
                            nc.scalar.activation(
                                out=et,
                                in_=s_ps,
                                func=Act.Exp,
                                bias=eb_t[:, kb * HPC + h : kb * HPC + h + 1],
                                scale=0.125,
                            )
                            nc.tensor.matmul(
                                ctx_ps,
                                lhsT=v_t[kb].rearrange(
                                    "p (h c) -> p h c", h=HPC
                                )[:, h, :],
                                rhs=et,
                                start=(kb == 0),
                                stop=(kb == NKB - 1),
                            )
                        # normalize + emit
                        ctx_sb = ework.tile([65, QT], mybir.dt.float32, tag="ctxsb")
                        nc.scalar.copy(out=ctx_sb, in_=ctx_ps)
                        for c in range(4):
                            tp = psum_tp.tile([128, 65], mybir.dt.float32, tag="tp")
                            nc.tensor.transpose(
                                tp,
                                ctx_sb[:, c * 128 : (c + 1) * 128],
                                idf_t[:65, :65],
                            )
                            rc = ework.tile([128, 1], mybir.dt.float32, tag="rc")
                            nc.vector.reciprocal(out=rc, in_=tp[:, DH : DH + 1])
                            ot = ework.tile([128, DH], mybir.dt.float32, tag="ot")
                            nc.vector.tensor_scalar(
                                out=ot,
                                in0=tp[:, 0:DH],
                                scalar1=rc,
                                scalar2=None,
                                op0=Alu.mult,
                            )
                            nc.sync.dma_start(
                                out=y[
                                    qt * QT + c * 128 : qt * QT + (c + 1) * 128,
                                    hq : hq + DH,
                                ],
                                in_=ot,
                            )
    nc.compile()
    return nc


def _get_nc():
    if "nc" not in _cache:
        _cache["nc"] = _build()
    return _cache["nc"]


def _host_prep(hidden_states, attention_mask, relative_position_matrix,
               Wq, bq, Wk, bk, Wv, bv, rel_bias):
    bf = ml_dtypes.bfloat16
    hs = np.asarray(hidden_states, np.float32)
    am = np.asarray(attention_mask, np.float32)
    rpm = np.asarray(relative_position_matrix, np.int32)
    Wq, bq = np.asarray(Wq, np.float32), np.asarray(bq, np.float32)
    Wk, bk = np.asarray(Wk, np.float32), np.asarray(bk, np.float32)
    Wv, bv = np.asarray(Wv, np.float32), np.asarray(bv, np.float32)
    rel_bias = np.asarray(rel_bias, np.float32)

    # PWL basis matrix on x=0..7: columns [1, x, relu(x-1)..relu(x-6)]
    xs = np.arange(8, dtype=np.float64)
    M = np.stack(
        [np.ones(8), xs] + [np.maximum(xs - j, 0.0) for j in range(1, 7)], axis=1
    )
    sol = np.linalg.solve(M, rel_bias[:8, :].astype(np.float64))  # [8, 12]

    # per-batch xT_aug
    xT_all = []
    for b in range(B):
        xa = np.zeros((DAUG, S), np.float32)
        xa[:D, :] = hs[b].T
        xa[D, :] = 1.0
        xT_all.append(xa.astype(bf))

    def wprep(W, bias, hg):
        wa = np.zeros((DAUG, HPC * DH), np.float32)
        sl = slice(hg * HPC * DH, (hg + 1) * HPC * DH)
        wa[:D, :] = W[sl, :].T
        wa[D, :] = bias[sl]
        return wa.astype(bf)

    in_maps = []
    for c in range(8):
        b, hg = c // 2, c % 2
        co = np.zeros((128, HPC * 8), np.float32)
        ebm = np.zeros((128, NKB * HPC), np.float32)
        mask_k = am[b, 0, 0, :]
        for h in range(HPC):
            gh = hg * HPC + h
            a_h = sol[0, gh]
            co[:, h * 8 + 0] = 8.0 * sol[1, gh]
            for j in range(1, 7):
                co[:, h * 8 + j] = 8.0 * sol[1 + j, gh]
            for kb in range(NKB):
                ebm[:, kb * HPC + h] = a_h + mask_k[kb * 128 : (kb + 1) * 128]
        in_maps.append({
            "xT": np.ascontiguousarray(xT_all[b]),
            "wqT": wprep(Wq, bq, hg),
            "wkT": wprep(Wk, bk, hg),
            "wvT": wprep(Wv, bv, hg),
            "rel": np.ascontiguousarray(rpm[b]),
            "coefs": co,
            "eb": ebm,
            "idb": np.eye(128, dtype=np.float32).astype(bf),
            "idf": np.eye(128, dtype=np.float32),
        })
    return in_maps


def kernel(**inputs):
    from concourse.bass_utils import run_bass_kernel_spmd

    nc = _get_nc()
    in_maps = _host_prep(**inputs)
    res = run_bass_kernel_spmd(nc, in_maps, list(range(8)))
    out = np.zeros((B, S, D), np.float32)
    for c in range(8):
        b, hg = c // 2, c % 2
        out[b, :, hg * HPC * DH : (hg + 1) * HPC * DH] = res.results[c]["y"]
    return out


if __name__ == "__main__":
    sys.path.insert(0, "/root/problem")
    import reference

    inputs = {k: np.asarray(v) for k, v in reference.setup_inputs().items()}
    expected = np.asarray(reference.reference(**inputs))
    actual = kernel(**inputs)
    err = np.abs(actual - expected)
    denom = np.abs(expected).max()
    print("max abs err:", err.max(), "scale:", denom)
    print("rel err (fro):",
          np.linalg.norm((actual - expected).ravel())
          / np.linalg.norm(expected.ravel()))
